# revision 41
# baseline (speedup 1.0000x reference)
"""Causal multi-head attention block (QKV proj -> causal attention -> out proj)
for Trainium2, distributed over 8 NeuronCores.

Sharding: core c handles batch b = c//2 and head-group g = c%2 (8 of 16 heads).
Each core computes qkv for its group's columns of w_attn, runs causal attention
for its 8 heads, and multiplies by its group's rows of w_proj, producing a
partial y[b]. The host sums the two partials per batch and adds b_proj.

All device matmuls run in fp32r (full-rate fp32 streaming mode, ~2e-4 rel
err). The kernel works in transposed layouts end-to-end (host passes x[b].T,
device returns y[b].T) so no on-device transposes are needed:
  q^T,k^T = w_{q,k}^T-chunks @ x^T      [cols, tok]
  s^T     = k_h^T-chunks    @ q_h^T     [k_tok, q_tok]  (exp on ACT -> p^T,
            causal tri-mask multiply on the idle GPSIMD engine)
  out^T   = [v_h | 1x32]^T  @ p^T       [96, q_tok]: v is padded with 32 ones
            columns so the pv matmul lands the softmax denominators
            REPLICATED on psum partitions 64..95 -- normalization is then
            just a fast-approx reciprocal + two multiplies on DVE, with no
            partition-broadcast op at all
  y^T     = w_proj-chunks   @ out_norm^T
Emission order pipelines phases: chunked input DMAs, v first, then per
head-pair q/k projection + qc=0 attention, then qc=1 attention with the
token-half-0 output projection interleaved into the exp-bound gaps.
Measured ~150 us per-core on TRN2 (8 cores run the 4-batch problem).
"""

import math
import sys

import numpy as np

if "/opt/trn_rl_repo" not in sys.path:
    sys.path.insert(0, "/opt/trn_rl_repo")

B, S, D = 4, 1024, 1024
H = 16
HPG = 8              # heads per group (2 groups of 8)
hd = D // H          # 64
GC = HPG * hd        # 512 cols per group for each of q,k,v
P = 128
DC = D // P          # 8 contraction chunks
NEG = None           # masking is multiplicative (exact zero), not additive

_CACHE = {}


def _build(repeat=1, no_norm=False, wp_stream=False, ones_cols=32, fast_recip=True, no_mask=False, copy_evict=False, mask_mm=False, mask_pool=True, qkv_evict_act=True, mask_split=False, pt_bufs=6, v_evict_act=False, proj0_act=False, dma_quad=False):
    import concourse.mybir as mybir
    import concourse.tile as tile
    from concourse import bacc
    from concourse.masks import make_upper_triangular

    f32 = mybir.dt.float32
    f32r = mybir.dt.float32r
    Exp = mybir.ActivationFunctionType.Exp
    mult = mybir.AluOpType.mult

    nc = bacc.Bacc("TRN2", target_bir_lowering=False, debug=False, num_devices=8)
    xT = nc.dram_tensor("xT", [D, S], f32r, kind="ExternalInput").ap()
    wq = nc.dram_tensor("wq", [D, GC], f32r, kind="ExternalInput").ap()
    wk = nc.dram_tensor("wk", [D, GC], f32r, kind="ExternalInput").ap()
    wv = nc.dram_tensor("wv", [D, GC], f32r, kind="ExternalInput").ap()
    wp = nc.dram_tensor("wp", [GC, D], f32r, kind="ExternalInput").ap()
    yT = nc.dram_tensor("yT", [D, S], f32, kind="ExternalOutput").ap()

    scale = 1.0 / math.sqrt(hd)

    with tile.TileContext(nc) as tc:
        with tc.tile_pool(name="const", bufs=1) as const, \
             tc.tile_pool(name="big", bufs=1) as big, \
             tc.tile_pool(name="pt", bufs=pt_bufs) as ptp, \
             tc.tile_pool(name="small", bufs=3) as small, \
             tc.tile_pool(name="wps", bufs=4) as wps, \
             tc.tile_pool(name="yt", bufs=2) as ytp, \
             tc.tile_pool(name="ps", bufs=8, space="PSUM") as ps:

          for _rep in range(repeat):
            tri = const.tile([P, P], f32, tag="tri")      # keep iff k_local <= q_local
            make_upper_triangular(nc, tri[:], val=1.0, diag=True)
            ones_col = const.tile([P, 1], f32, tag="ones_col")
            nc.any.memset(ones_col[:], 1.0)
            if mask_mm:
                # causal mask as a matmul: triA.T @ id accumulates -1e9 onto
                # masked score entries so exp underflows to exact 0
                from concourse.masks import make_identity
                triA_f = const.tile([P, P], f32, tag="triA_f")
                make_upper_triangular(nc, triA_f[:], val=-1e9, diag=False)
                triA = const.tile([P, P], f32r, tag="triA")
                nc.vector.tensor_copy(out=triA[:], in_=triA_f[:])
                id_f = const.tile([P, P], f32, tag="id_f")
                make_identity(nc, id_f[:])
                id_r = const.tile([P, P], f32r, tag="id_r")
                nc.vector.tensor_copy(out=id_r[:], in_=id_f[:])

            # chunked input loads so compute can start on early chunks
            xt = big.tile([P, DC, S], f32r, tag="xt")
            xTr = xT.rearrange("(dc p) t -> p dc t", p=P)
            wqt = big.tile([P, DC, GC], f32r, tag="wq")
            wqr = wq.rearrange("(dc p) c -> p dc c", p=P)
            wkt = big.tile([P, DC, GC], f32r, tag="wk")
            wkr = wk.rearrange("(dc p) c -> p dc c", p=P)
            wvt = big.tile([P, DC, GC], f32r, tag="wv")
            wvr = wv.rearrange("(dc p) c -> p dc c", p=P)
            if dma_quad:
                # all four streams advance together: every arriving d-chunk
                # unlocks that chunk's q/k/v matmuls across all groups
                for dc in range(DC):
                    nc.sync.dma_start(xt[:, dc:dc + 1, :], xTr[:, dc:dc + 1, :])
                    nc.sync.dma_start(wqt[:, dc:dc + 1, :], wqr[:, dc:dc + 1, :])
                    nc.sync.dma_start(wkt[:, dc:dc + 1, :], wkr[:, dc:dc + 1, :])
                    nc.sync.dma_start(wvt[:, dc:dc + 1, :], wvr[:, dc:dc + 1, :])
            else:
                for dc in range(DC):
                    nc.sync.dma_start(xt[:, dc:dc + 1, :], xTr[:, dc:dc + 1, :])
                    nc.sync.dma_start(wvt[:, dc:dc + 1, :], wvr[:, dc:dc + 1, :])
                for dc in range(DC):
                    nc.sync.dma_start(wqt[:, dc:dc + 1, :], wqr[:, dc:dc + 1, :])
                    nc.sync.dma_start(wkt[:, dc:dc + 1, :], wkr[:, dc:dc + 1, :])

            if not wp_stream:
                wpt_c = big.tile([P, GC // P, D], f32r, tag="wp")
                wpr = wp.rearrange("(cc p) o -> p cc o", p=P)
                for cc in range(GC // P):
                    nc.sync.dma_start(wpt_c[:, cc:cc + 1, :], wpr[:, cc:cc + 1, :])

            # q^T/k^T for the group: [col(128), chunk, tok]; chunks 0-3 = q, 4-7 = k
            qkt = big.tile([P, 2 * GC // P, S], f32r, tag="qkt")
            # v padded with 64 ones columns per head: the pv matmul then
            # lands sum(p) replicated on psum partitions 64..127, so softmax
            # normalization needs no partition broadcast at all.
            vaug = big.tile([P, S // P, HPG, hd + ones_cols], f32r, tag="vaug")
            nc.vector.tensor_copy(
                out=vaug[:, :, :, hd:hd + ones_cols],
                in_=ones_col[:].to_broadcast([P, S // P, HPG, ones_cols]))
            # normalized attention output ^T: [chan(128), chan_chunk, tok]
            outt = big.tile([P, GC // P, S], f32r, tag="outt")

            # ---- qkv projections ----
            def qk_group(cc8, t5):
                src = wqt if cc8 < 4 else wkt
                cbase = (cc8 % 4) * P
                acc = ps.tile([P, 512], f32, tag="ps")
                for dc in range(DC):
                    nc.tensor.matmul(
                        acc[:],
                        src[:, dc, cbase:cbase + P],
                        xt[:, dc, t5 * 512:(t5 + 1) * 512],
                        start=(dc == 0), stop=(dc == DC - 1),
                    )
                if qkv_evict_act:
                    nc.scalar.copy(qkt[:, cc8, t5 * 512:(t5 + 1) * 512], acc[:])
                else:
                    nc.vector.tensor_copy(
                        out=qkt[:, cc8, t5 * 512:(t5 + 1) * 512], in_=acc[:])

            def v_group(t8):
                acc = ps.tile([P, 512], f32, tag="ps")
                for dc in range(DC):
                    nc.tensor.matmul(
                        acc[:],
                        xt[:, dc, t8 * P:(t8 + 1) * P],
                        wvt[:, dc, :],
                        start=(dc == 0), stop=(dc == DC - 1),
                    )
                if v_evict_act:
                    nc.scalar.copy(vaug[:, t8, :, 0:hd],
                                   acc[:].rearrange("p (h j) -> p h j", h=HPG))
                else:
                    nc.vector.tensor_copy(
                        out=vaug[:, t8, :, 0:hd],
                        in_=acc[:].rearrange("p (h j) -> p h j", h=HPG))


            # ---- output projection groups (emitted per token-half below) ----
            def proj_group(t5, oc):
                acc = ps.tile([P, 512], f32, tag="ps")
                for cc in range(GC // P):
                    if wp_stream:
                        wpt = wps.tile([P, P], f32r, tag="wps")
                        nc.sync.dma_start(
                            wpt[:], wp[cc * P:(cc + 1) * P, oc * P:(oc + 1) * P])
                        lhs = wpt[:]
                    else:
                        lhs = wpt_c[:, cc, oc * P:(oc + 1) * P]
                    nc.tensor.matmul(
                        acc[:],
                        lhs,
                        outt[:, cc, t5 * 512:(t5 + 1) * 512],
                        start=(cc == 0), stop=(cc == GC // P - 1),
                    )
                yt = ytp.tile([P, 512], f32, tag="yt")
                if t5 == 0 and not proj0_act:
                    nc.vector.tensor_copy(out=yt[:], in_=acc[:])  # ACT busy w/ exp
                else:
                    nc.scalar.copy(yt[:], acc[:])                 # tail: ACT idle
                nc.sync.dma_start(
                    yT[oc * P:(oc + 1) * P, t5 * 512:(t5 + 1) * 512], yt[:])

            # ---- causal attention, transposed layouts ----
            def attn(h, qc):
                    prow = 64 * (h % 2)
                    qh = qkt[prow:prow + hd, h // 2, :]
                    kh = qkt[prow:prow + hd, 4 + h // 2, :]
                    acc = ps.tile([P, 512], f32, tag="ps")
                    # k-blocks: diagonal p=0 first (full width, start=True),
                    # then fully-below blocks, then partial-width diagonals
                    order = [4 * qc] + list(range(4 * qc)) + \
                        [4 * qc + p for p in (1, 2, 3)]
                    for i, kb in enumerate(order):
                        p_off = kb - 4 * qc            # >=0 on diagonal blocks
                        start_col = max(0, p_off) * P
                        width = 512 - start_col
                        diag = p_off >= 0 and not no_mask
                        sp = ps.tile([P, 512], f32, tag="ps")
                        nc.tensor.matmul(
                            sp[:, :width],
                            kh[:, kb * P:(kb + 1) * P],
                            qh[:, qc * 512 + start_col:(qc + 1) * 512],
                            start=True, stop=not (diag and mask_mm),
                        )
                        if diag and mask_mm:
                            nc.tensor.matmul(sp[:, 0:P], triA[:], id_r[:],
                                             start=False, stop=True)
                        pt = ptp.tile([P, 512], f32r, tag="pt")
                        if copy_evict:   # timing probe: DVE copy instead of exp
                            nc.vector.tensor_copy(out=pt[:, :width],
                                                  in_=sp[:, :width])
                        else:
                            nc.scalar.activation(pt[:, :width], sp[:, :width],
                                                 Exp, scale=scale)
                        if diag and not mask_mm:        # triangular mask part
                            if mask_split:
                                eng = nc.gpsimd if (kb % 2 == 0) else nc.vector
                            else:
                                eng = nc.gpsimd if mask_pool else nc.vector
                            eng.tensor_tensor(
                                pt[:, 0:P], pt[:, 0:P], tri[:], mult)
                        nc.tensor.matmul(
                            acc[:hd + ones_cols, start_col:512],
                            vaug[:, kb, h, :],
                            pt[:, :width],
                            start=(i == 0), stop=(i == len(order) - 1),
                        )
                    # normalize: psum rows 64..127 hold rowsum replicated
                    # 64x (from vaug's ones block) -> reciprocal + multiply
                    if no_norm:
                        nc.vector.tensor_copy(
                            out=outt[prow:prow + hd, h // 2,
                                     qc * 512:(qc + 1) * 512],
                            in_=acc[0:hd, :])
                    else:
                        rsb = small.tile([ones_cols, 512], f32, tag="rsb")
                        if fast_recip:
                            # ~18-bit reciprocal, ~5x faster than the exact
                            # InstReciprocal; denominators are benign sums.
                            # (custom-DVE op misreads PSUM: stage via SBUF)
                            rss = small.tile([ones_cols, 512], f32, tag="rss")
                            nc.vector.tensor_copy(
                                out=rss[:], in_=acc[hd:hd + ones_cols, :])
                            nc.vector.reciprocal_approx_fast(
                                out=rsb[:], in_=rss[:])
                        else:
                            nc.vector.reciprocal(rsb[:], acc[hd:hd + ones_cols, :])
                        for s0 in range(0, hd, ones_cols):
                            n0 = min(ones_cols, hd - s0)
                            nc.vector.tensor_tensor(
                                outt[prow + s0:prow + s0 + n0, h // 2,
                                     qc * 512:(qc + 1) * 512],
                                acc[s0:s0 + n0, :], rsb[:n0, :], mult)

            # ---- emission schedule ----
            # v first (wv streams right behind xT), then per head-pair its
            # q/k projections followed immediately by that pair's qc=0
            # attention, so exp (ACT) overlaps the remaining projections.
            for t8 in range(S // P):
                v_group(t8)
            for hp in range(GC // P):
                qk_group(hp, 0)
                qk_group(hp, 1)
                qk_group(4 + hp, 0)
                qk_group(4 + hp, 1)
                attn(2 * hp, 0)
                attn(2 * hp + 1, 0)
            # qc=1 attention with token-half-0 projection interleaved: proj
            # matmuls fill PE gaps left by the exp-bound attention pacing,
            # and the first 2MB of output streams out early
            for hp in range(GC // P):
                attn(2 * hp, 1)
                proj_group(0, 2 * hp)
                attn(2 * hp + 1, 1)
                proj_group(0, 2 * hp + 1)
            for oc in range(D // P):
                proj_group(1, oc)

    nc.compile()
    return nc


def _get_nc(repeat=1, **kw):
    key = ("nc", repeat, tuple(sorted(kw.items())))
    if key not in _CACHE:
        _CACHE[key] = _build(repeat, **kw)
    return _CACHE[key]


def make_in_maps(x, w_attn):
    """Per-core input shards (core c -> batch c//2, head-group c%2)."""
    in_maps = []
    xTs = [np.ascontiguousarray(x[b].T) for b in range(B)]
    for c in range(8):
        b, g = divmod(c, 2)
        in_maps.append({
            "xT": xTs[b],
            "wq": np.ascontiguousarray(w_attn[:, g * GC:(g + 1) * GC]),
            "wk": np.ascontiguousarray(w_attn[:, D + g * GC:D + (g + 1) * GC]),
            "wv": np.ascontiguousarray(w_attn[:, 2 * D + g * GC:2 * D + (g + 1) * GC]),
            "wp": None,  # filled below
        })
    return in_maps


def kernel(x, w_attn, b_attn, w_proj, b_proj):
    x = np.asarray(x, dtype=np.float32)
    w_attn = np.asarray(w_attn, dtype=np.float32)
    b_attn = np.asarray(b_attn, dtype=np.float32)
    w_proj = np.asarray(w_proj, dtype=np.float32)
    b_proj = np.asarray(b_proj, dtype=np.float32)

    if np.any(b_attn):
        # Spec guarantees b_attn == 0 (fill: zeros); exact fallback if not.
        return _numpy_reference(x, w_attn, b_attn, w_proj, b_proj)

    from concourse.bass_utils import run_bass_kernel_spmd

    nc = _get_nc()
    in_maps = make_in_maps(x, w_attn)
    for c in range(8):
        g = c % 2
        in_maps[c]["wp"] = np.ascontiguousarray(w_proj[g * GC:(g + 1) * GC, :])

    res = run_bass_kernel_spmd(nc, in_maps, core_ids=list(range(8)))
    y = np.empty((B, S, D), np.float32)
    for b in range(B):
        y[b] = res.results[2 * b]["yT"].T + res.results[2 * b + 1]["yT"].T + b_proj
    return y


def _numpy_reference(x, w_attn, b_attn, w_proj, b_proj):
    qkv = x @ w_attn + b_attn
    q, k, v = np.split(qkv, 3, axis=-1)

    def heads(t):
        return t.reshape(B, S, H, hd).transpose(0, 2, 1, 3)

    q, k, v = heads(q), heads(k), heads(v)
    scores = np.einsum("bhqd,bhkd->bhqk", q, k) / np.sqrt(np.float32(hd))
    causal = np.tril(np.ones((S, S), dtype=bool))[None, None]
    scores = np.where(causal, scores, -1e9)
    scores -= scores.max(axis=-1, keepdims=True)
    attn = np.exp(scores)
    attn /= attn.sum(axis=-1, keepdims=True)
    out = np.einsum("bhqk,bhkd->bhqd", attn, v)
    out = out.transpose(0, 2, 1, 3).reshape(B, S, D)
    return out @ w_proj + b_proj


# revision 44
# speedup vs baseline: 2.7868x; 2.7868x over previous
"""Causal multi-head attention block (QKV proj -> causal attention -> out proj)
for Trainium2, distributed over 8 NeuronCores.

Sharding: core c handles batch b = c//2 and head-group g = c%2 (8 of 16 heads).
Each core computes qkv for its group's columns of w_attn, runs causal attention
for its 8 heads, and multiplies by its group's rows of w_proj, producing a
partial y[b]. The host sums the two partials per batch and adds b_proj.

All device matmuls run in fp32r (full-rate fp32 streaming mode, ~2e-4 rel
err). The kernel works in transposed layouts end-to-end (host passes x[b].T,
device returns y[b].T) so no on-device transposes are needed:
  q^T,k^T = w_{q,k}^T-chunks @ x^T      [cols, tok]
  s^T     = k_h^T-chunks    @ q_h^T     [k_tok, q_tok]  (exp on ACT -> p^T,
            causal tri-mask multiply on the idle GPSIMD engine)
  out^T   = [v_h | 1x32]^T  @ p^T       [96, q_tok]: v is padded with 32 ones
            columns so the pv matmul lands the softmax denominators
            REPLICATED on psum partitions 64..95 -- normalization is then
            just a fast-approx reciprocal + two multiplies on DVE, with no
            partition-broadcast op at all
  y^T     = w_proj-chunks   @ out_norm^T
Emission order pipelines phases: chunked input DMAs, v first, then per
head-pair q/k projection + qc=0 attention, then qc=1 attention with the
token-half-0 output projection interleaved into the exp-bound gaps.
Measured ~150 us per-core on TRN2 (8 cores run the 4-batch problem).
"""

import math
import sys

import numpy as np

if "/opt/trn_rl_repo" not in sys.path:
    sys.path.insert(0, "/opt/trn_rl_repo")

B, S, D = 4, 1024, 1024
H = 16
HPG = 8              # heads per group (2 groups of 8)
hd = D // H          # 64
GC = HPG * hd        # 512 cols per group for each of q,k,v
P = 128
DC = D // P          # 8 contraction chunks
NEG = None           # masking is multiplicative (exact zero), not additive

_CACHE = {}


def _build(repeat=1, no_norm=False, wp_stream=False, ones_cols=32, fast_recip=True, no_mask=False, copy_evict=False, mask_mm=False, mask_pool=True, qkv_evict_act=True, mask_split=False, pt_bufs=6, v_evict_act=False, proj0_act=False, dma_quad=False, skew=False):
    import concourse.mybir as mybir
    import concourse.tile as tile
    from concourse import bacc
    from concourse.masks import make_upper_triangular

    f32 = mybir.dt.float32
    f32r = mybir.dt.float32r
    Exp = mybir.ActivationFunctionType.Exp
    mult = mybir.AluOpType.mult

    nc = bacc.Bacc("TRN2", target_bir_lowering=False, debug=False, num_devices=8)
    xT = nc.dram_tensor("xT", [D, S], f32r, kind="ExternalInput").ap()
    wq = nc.dram_tensor("wq", [D, GC], f32r, kind="ExternalInput").ap()
    wk = nc.dram_tensor("wk", [D, GC], f32r, kind="ExternalInput").ap()
    wv = nc.dram_tensor("wv", [D, GC], f32r, kind="ExternalInput").ap()
    wp = nc.dram_tensor("wp", [GC, D], f32r, kind="ExternalInput").ap()
    yT = nc.dram_tensor("yT", [D, S], f32, kind="ExternalOutput").ap()

    scale = 1.0 / math.sqrt(hd)

    with tile.TileContext(nc) as tc:
        with tc.tile_pool(name="const", bufs=1) as const, \
             tc.tile_pool(name="big", bufs=1) as big, \
             tc.tile_pool(name="pt", bufs=pt_bufs) as ptp, \
             tc.tile_pool(name="small", bufs=3) as small, \
             tc.tile_pool(name="wps", bufs=4) as wps, \
             tc.tile_pool(name="yt", bufs=2) as ytp, \
             tc.tile_pool(name="ps", bufs=8, space="PSUM") as ps:

          for _rep in range(repeat):
            tri = const.tile([P, P], f32, tag="tri")      # keep iff k_local <= q_local
            make_upper_triangular(nc, tri[:], val=1.0, diag=True)
            ones_col = const.tile([P, 1], f32, tag="ones_col")
            nc.any.memset(ones_col[:], 1.0)
            if mask_mm:
                # causal mask as a matmul: triA.T @ id accumulates -1e9 onto
                # masked score entries so exp underflows to exact 0
                from concourse.masks import make_identity
                triA_f = const.tile([P, P], f32, tag="triA_f")
                make_upper_triangular(nc, triA_f[:], val=-1e9, diag=False)
                triA = const.tile([P, P], f32r, tag="triA")
                nc.vector.tensor_copy(out=triA[:], in_=triA_f[:])
                id_f = const.tile([P, P], f32, tag="id_f")
                make_identity(nc, id_f[:])
                id_r = const.tile([P, P], f32r, tag="id_r")
                nc.vector.tensor_copy(out=id_r[:], in_=id_f[:])

            # chunked input loads so compute can start on early chunks
            xt = big.tile([P, DC, S], f32r, tag="xt")
            xTr = xT.rearrange("(dc p) t -> p dc t", p=P)
            wqt = big.tile([P, DC, GC], f32r, tag="wq")
            wqr = wq.rearrange("(dc p) c -> p dc c", p=P)
            wkt = big.tile([P, DC, GC], f32r, tag="wk")
            wkr = wk.rearrange("(dc p) c -> p dc c", p=P)
            wvt = big.tile([P, DC, GC], f32r, tag="wv")
            wvr = wv.rearrange("(dc p) c -> p dc c", p=P)
            if dma_quad:
                # all four streams advance together: every arriving d-chunk
                # unlocks that chunk's q/k/v matmuls across all groups
                for dc in range(DC):
                    nc.sync.dma_start(xt[:, dc:dc + 1, :], xTr[:, dc:dc + 1, :])
                    nc.sync.dma_start(wqt[:, dc:dc + 1, :], wqr[:, dc:dc + 1, :])
                    nc.sync.dma_start(wkt[:, dc:dc + 1, :], wkr[:, dc:dc + 1, :])
                    nc.sync.dma_start(wvt[:, dc:dc + 1, :], wvr[:, dc:dc + 1, :])
            else:
                for dc in range(DC):
                    nc.sync.dma_start(xt[:, dc:dc + 1, :], xTr[:, dc:dc + 1, :])
                    nc.sync.dma_start(wvt[:, dc:dc + 1, :], wvr[:, dc:dc + 1, :])
                for dc in range(DC):
                    nc.sync.dma_start(wqt[:, dc:dc + 1, :], wqr[:, dc:dc + 1, :])
                    nc.sync.dma_start(wkt[:, dc:dc + 1, :], wkr[:, dc:dc + 1, :])

            if not wp_stream:
                wpt_c = big.tile([P, GC // P, D], f32r, tag="wp")
                wpr = wp.rearrange("(cc p) o -> p cc o", p=P)
                for cc in range(GC // P):
                    nc.sync.dma_start(wpt_c[:, cc:cc + 1, :], wpr[:, cc:cc + 1, :])

            # q^T/k^T for the group: [col(128), chunk, tok]; chunks 0-3 = q, 4-7 = k
            qkt = big.tile([P, 2 * GC // P, S], f32r, tag="qkt")
            # v padded with 64 ones columns per head: the pv matmul then
            # lands sum(p) replicated on psum partitions 64..127, so softmax
            # normalization needs no partition broadcast at all.
            vaug = big.tile([P, S // P, HPG, hd + ones_cols], f32r, tag="vaug")
            nc.vector.tensor_copy(
                out=vaug[:, :, :, hd:hd + ones_cols],
                in_=ones_col[:].to_broadcast([P, S // P, HPG, ones_cols]))
            # normalized attention output ^T: [chan(128), chan_chunk, tok]
            outt = big.tile([P, GC // P, S], f32r, tag="outt")

            # ---- qkv projections ----
            def qk_group(cc8, t5):
                src = wqt if cc8 < 4 else wkt
                cbase = (cc8 % 4) * P
                acc = ps.tile([P, 512], f32, tag="ps")
                for dc in range(DC):
                    nc.tensor.matmul(
                        acc[:],
                        src[:, dc, cbase:cbase + P],
                        xt[:, dc, t5 * 512:(t5 + 1) * 512],
                        start=(dc == 0), stop=(dc == DC - 1),
                    )
                if qkv_evict_act:
                    nc.scalar.copy(qkt[:, cc8, t5 * 512:(t5 + 1) * 512], acc[:])
                else:
                    nc.vector.tensor_copy(
                        out=qkt[:, cc8, t5 * 512:(t5 + 1) * 512], in_=acc[:])

            def v_group(t8):
                acc = ps.tile([P, 512], f32, tag="ps")
                for dc in range(DC):
                    nc.tensor.matmul(
                        acc[:],
                        xt[:, dc, t8 * P:(t8 + 1) * P],
                        wvt[:, dc, :],
                        start=(dc == 0), stop=(dc == DC - 1),
                    )
                if v_evict_act:
                    nc.scalar.copy(vaug[:, t8, :, 0:hd],
                                   acc[:].rearrange("p (h j) -> p h j", h=HPG))
                else:
                    nc.vector.tensor_copy(
                        out=vaug[:, t8, :, 0:hd],
                        in_=acc[:].rearrange("p (h j) -> p h j", h=HPG))


            # ---- output projection groups (emitted per token-half below) ----
            def proj_group(t5, oc):
                acc = ps.tile([P, 512], f32, tag="ps")
                for cc in range(GC // P):
                    if wp_stream:
                        wpt = wps.tile([P, P], f32r, tag="wps")
                        nc.sync.dma_start(
                            wpt[:], wp[cc * P:(cc + 1) * P, oc * P:(oc + 1) * P])
                        lhs = wpt[:]
                    else:
                        lhs = wpt_c[:, cc, oc * P:(oc + 1) * P]
                    nc.tensor.matmul(
                        acc[:],
                        lhs,
                        outt[:, cc, t5 * 512:(t5 + 1) * 512],
                        start=(cc == 0), stop=(cc == GC // P - 1),
                    )
                yt = ytp.tile([P, 512], f32, tag="yt")
                if t5 == 0 and not proj0_act:
                    nc.vector.tensor_copy(out=yt[:], in_=acc[:])  # ACT busy w/ exp
                else:
                    nc.scalar.copy(yt[:], acc[:])                 # tail: ACT idle
                nc.sync.dma_start(
                    yT[oc * P:(oc + 1) * P, t5 * 512:(t5 + 1) * 512], yt[:])

            # ---- causal attention, transposed layouts ----
            def attn(h, qc):
                    prow = 64 * (h % 2)
                    qh = qkt[prow:prow + hd, h // 2, :]
                    kh = qkt[prow:prow + hd, 4 + h // 2, :]
                    acc = ps.tile([P, 512], f32, tag="ps")
                    # k-blocks: diagonal p=0 first (full width, start=True),
                    # then fully-below blocks, then partial-width diagonals
                    order = [4 * qc] + list(range(4 * qc)) + \
                        [4 * qc + p for p in (1, 2, 3)]
                    for i, kb in enumerate(order):
                        p_off = kb - 4 * qc            # >=0 on diagonal blocks
                        start_col = max(0, p_off) * P
                        width = 512 - start_col
                        diag = p_off >= 0 and not no_mask
                        sp = ps.tile([P, 512], f32, tag="ps")
                        nc.tensor.matmul(
                            sp[:, :width],
                            kh[:, kb * P:(kb + 1) * P],
                            qh[:, qc * 512 + start_col:(qc + 1) * 512],
                            start=True, stop=not (diag and mask_mm),
                        )
                        if diag and mask_mm:
                            nc.tensor.matmul(sp[:, 0:P], triA[:], id_r[:],
                                             start=False, stop=True)
                        pt = ptp.tile([P, 512], f32r, tag="pt")
                        if copy_evict:   # timing probe: DVE copy instead of exp
                            nc.vector.tensor_copy(out=pt[:, :width],
                                                  in_=sp[:, :width])
                        else:
                            nc.scalar.activation(pt[:, :width], sp[:, :width],
                                                 Exp, scale=scale)
                        if diag and not mask_mm:        # triangular mask part
                            if mask_split:
                                eng = nc.gpsimd if (kb % 2 == 0) else nc.vector
                            else:
                                eng = nc.gpsimd if mask_pool else nc.vector
                            eng.tensor_tensor(
                                pt[:, 0:P], pt[:, 0:P], tri[:], mult)
                        nc.tensor.matmul(
                            acc[:hd + ones_cols, start_col:512],
                            vaug[:, kb, h, :],
                            pt[:, :width],
                            start=(i == 0), stop=(i == len(order) - 1),
                        )
                    # normalize: psum rows 64..127 hold rowsum replicated
                    # 64x (from vaug's ones block) -> reciprocal + multiply
                    if no_norm:
                        nc.vector.tensor_copy(
                            out=outt[prow:prow + hd, h // 2,
                                     qc * 512:(qc + 1) * 512],
                            in_=acc[0:hd, :])
                    else:
                        rsb = small.tile([ones_cols, 512], f32, tag="rsb")
                        if fast_recip:
                            # ~18-bit reciprocal, ~5x faster than the exact
                            # InstReciprocal; denominators are benign sums.
                            # (custom-DVE op misreads PSUM: stage via SBUF)
                            rss = small.tile([ones_cols, 512], f32, tag="rss")
                            nc.vector.tensor_copy(
                                out=rss[:], in_=acc[hd:hd + ones_cols, :])
                            nc.vector.reciprocal_approx_fast(
                                out=rsb[:], in_=rss[:])
                        else:
                            nc.vector.reciprocal(rsb[:], acc[hd:hd + ones_cols, :])
                        for s0 in range(0, hd, ones_cols):
                            n0 = min(ones_cols, hd - s0)
                            nc.vector.tensor_tensor(
                                outt[prow + s0:prow + s0 + n0, h // 2,
                                     qc * 512:(qc + 1) * 512],
                                acc[s0:s0 + n0, :], rsb[:n0, :], mult)

            # ---- emission schedule ----
            # v first (wv streams right behind xT), then per head-pair its
            # q/k projections followed immediately by that pair's qc=0
            # attention, so exp (ACT) overlaps the remaining projections.
            for t8 in range(S // P):
                v_group(t8)
            if skew:
                # attention emitted one pair behind its q/k projections: the
                # static scheduler then has the next pair's independent
                # matmuls adjacent to this pair's exp-chain stalls
                for hp in range(GC // P):
                    qk_group(hp, 0)
                    qk_group(hp, 1)
                    qk_group(4 + hp, 0)
                    qk_group(4 + hp, 1)
                    if hp > 0:
                        attn(2 * (hp - 1), 0)
                        attn(2 * (hp - 1) + 1, 0)
                attn(2 * (GC // P - 1), 0)
                attn(2 * (GC // P - 1) + 1, 0)
            else:
                for hp in range(GC // P):
                    qk_group(hp, 0)
                    qk_group(hp, 1)
                    qk_group(4 + hp, 0)
                    qk_group(4 + hp, 1)
                    attn(2 * hp, 0)
                    attn(2 * hp + 1, 0)
            # qc=1 attention with token-half-0 projection interleaved: proj
            # matmuls fill PE gaps left by the exp-bound attention pacing,
            # and the first 2MB of output streams out early
            for hp in range(GC // P):
                attn(2 * hp, 1)
                proj_group(0, 2 * hp)
                attn(2 * hp + 1, 1)
                proj_group(0, 2 * hp + 1)
            for oc in range(D // P):
                proj_group(1, oc)

    nc.compile()
    return nc


def _get_nc(repeat=1, **kw):
    key = ("nc", repeat, tuple(sorted(kw.items())))
    if key not in _CACHE:
        _CACHE[key] = _build(repeat, **kw)
    return _CACHE[key]


def make_in_maps(x, w_attn):
    """Per-core input shards (core c -> batch c//2, head-group c%2)."""
    in_maps = []
    xTs = [np.ascontiguousarray(x[b].T) for b in range(B)]
    for c in range(8):
        b, g = divmod(c, 2)
        in_maps.append({
            "xT": xTs[b],
            "wq": np.ascontiguousarray(w_attn[:, g * GC:(g + 1) * GC]),
            "wk": np.ascontiguousarray(w_attn[:, D + g * GC:D + (g + 1) * GC]),
            "wv": np.ascontiguousarray(w_attn[:, 2 * D + g * GC:2 * D + (g + 1) * GC]),
            "wp": None,  # filled below
        })
    return in_maps


def kernel(x, w_attn, b_attn, w_proj, b_proj):
    x = np.asarray(x, dtype=np.float32)
    w_attn = np.asarray(w_attn, dtype=np.float32)
    b_attn = np.asarray(b_attn, dtype=np.float32)
    w_proj = np.asarray(w_proj, dtype=np.float32)
    b_proj = np.asarray(b_proj, dtype=np.float32)

    if np.any(b_attn):
        # Spec guarantees b_attn == 0 (fill: zeros); exact fallback if not.
        return _numpy_reference(x, w_attn, b_attn, w_proj, b_proj)

    in_maps = make_in_maps(x, w_attn)
    for c in range(8):
        g = c % 2
        in_maps[c]["wp"] = np.ascontiguousarray(w_proj[g * GC:(g + 1) * GC, :])

    results = _run_cached(in_maps)
    y = np.empty((B, S, D), np.float32)
    for b in range(B):
        y[b] = results[2 * b]["yT"].T + results[2 * b + 1]["yT"].T + b_proj
    return y


def _run_cached(in_maps):
    """Execute the compiled module on 8 cores; the jitted PJRT runner is
    built once and reused so repeated kernel() calls skip retracing."""
    import jax
    from jax.sharding import Mesh, NamedSharding, PartitionSpec
    from jax.experimental.shard_map import shard_map
    import concourse.mybir as mybir
    from concourse.bass2jax import (_bass_exec_p, install_neuronx_cc_hook,
                                    partition_id_tensor)

    if "runner" not in _CACHE:
        install_neuronx_cc_hook()
        nc = _get_nc()
        partition_name = (nc.partition_id_tensor.name
                          if nc.partition_id_tensor else None)
        in_names, out_names, out_avals, zero_outs = [], [], [], []
        for alloc in nc.m.functions[0].allocations:
            if not isinstance(alloc, mybir.MemoryLocationSet):
                continue
            name = alloc.memorylocations[0].name
            if alloc.kind == "ExternalInput":
                if name != partition_name:
                    in_names.append(name)
            elif alloc.kind == "ExternalOutput":
                shape = tuple(alloc.tensor_shape)
                dtype = mybir.dt.np(alloc.dtype)
                out_names.append(name)
                out_avals.append(jax.core.ShapedArray(shape, dtype))
                zero_outs.append(np.zeros((8 * shape[0], *shape[1:]), dtype))
        all_in_names = list(in_names) + list(out_names)
        if partition_name is not None:
            all_in_names.append(partition_name)

        def _body(*args):
            operands = list(args)
            if partition_name is not None:
                operands.append(partition_id_tensor())
            return tuple(_bass_exec_p.bind(
                *operands,
                out_avals=tuple(out_avals),
                in_names=tuple(all_in_names),
                out_names=tuple(out_names),
                lowering_input_output_aliases=(),
                sim_require_finite=True,
                sim_require_nnan=True,
                nc=nc,
            ))

        devices = jax.devices()[:8]
        mesh = Mesh(np.asarray(devices), ("core",))
        n_ops = len(in_names) + len(out_names)
        fn = jax.jit(shard_map(
            _body, mesh=mesh,
            in_specs=(PartitionSpec("core"),) * n_ops,
            out_specs=(PartitionSpec("core"),) * len(out_names),
            check_rep=False), keep_unused=True)
        shard = NamedSharding(mesh, PartitionSpec("core"))
        zeros_dev = [jax.device_put(z, shard) for z in zero_outs]
        _CACHE["runner"] = (fn, in_names, out_names, zeros_dev, shard)

    fn, in_names, out_names, zeros_dev, shard = _CACHE["runner"]
    import jax
    concat_in = [np.concatenate([np.asarray(in_maps[c][n]) for c in range(8)],
                                axis=0) for n in in_names]
    dev_in = [jax.device_put(a, shard) for a in concat_in]
    out_arrs = fn(*dev_in, *zeros_dev)
    results = []
    for c in range(8):
        results.append({
            name: np.asarray(out_arrs[i]).reshape(8, -1, 1024)[c]
            for i, name in enumerate(out_names)})
    return results


def _numpy_reference(x, w_attn, b_attn, w_proj, b_proj):
    qkv = x @ w_attn + b_attn
    q, k, v = np.split(qkv, 3, axis=-1)

    def heads(t):
        return t.reshape(B, S, H, hd).transpose(0, 2, 1, 3)

    q, k, v = heads(q), heads(k), heads(v)
    scores = np.einsum("bhqd,bhkd->bhqk", q, k) / np.sqrt(np.float32(hd))
    causal = np.tril(np.ones((S, S), dtype=bool))[None, None]
    scores = np.where(causal, scores, -1e9)
    scores -= scores.max(axis=-1, keepdims=True)
    attn = np.exp(scores)
    attn /= attn.sum(axis=-1, keepdims=True)
    out = np.einsum("bhqk,bhkd->bhqd", attn, v)
    out = out.transpose(0, 2, 1, 3).reshape(B, S, D)
    return out @ w_proj + b_proj
